# revision 6
# baseline (speedup 1.0000x reference)
"""GCN (3x GCNConv + MLP head + weighted BCE) on 8 Trainium2 NeuronCores.

Node (target) sharding across 8 cores, 12500 nodes/core (padded to 12544).
Per layer:
  Phase A: m_loc = h_shard @ W on PE (node-major psum tiles) -> DRAM
  AllGather m_loc -> m_full [100352, 64] f32 (padded shard stride 12544)
  Phase C: per target tile (128 nodes) accumulate sum_e norm[e]*m[src[e]]
    via dma_gather (custom ucode gather; int16 idx into 25088-row quadrant
    slices of m_full) + DVE one-hot selection matrices + PE matmul in PSUM.
Head MLP + weighted-BCE partial sums on device; host combines the scalars.
"""
import numpy as np

N_NODES = 100000
N_FEAT = 128
HID = 64
NCORES = 8
NSH_V = 12500          # valid nodes per shard
NSH = 12544            # padded shard (98 * 128)
NT = 98                # target tiles per shard
NFULL = NSH * NCORES   # 100352
QROWS = NFULL // 4     # 25088 rows per gather-table slice (int16-safe)
P = 128
MAXCH_OP = 8           # chunks per dma_gather op (8*128 = 1024 idx)

_cache = {}


def _host_prep(x, edge_index, labels):
    row = np.asarray(edge_index[0], dtype=np.int64)
    col = np.asarray(edge_index[1], dtype=np.int64)
    loop = np.arange(N_NODES, dtype=np.int64)
    src = np.concatenate([row, loop])
    dst = np.concatenate([col, loop])
    deg = np.bincount(dst, minlength=N_NODES).astype(np.float32)
    dinv = (1.0 / np.sqrt(deg)).astype(np.float32)
    norm = (dinv[src] * dinv[dst]).astype(np.float32)

    rank = dst // NSH_V
    gp_src = (src // NSH_V) * NSH + (src % NSH_V)
    q = gp_src // QROWS
    lidx = gp_src % QROWS
    col_local = dst - rank * NSH_V
    tile_of = col_local // P
    col_rel = (col_local % P).astype(np.float32)

    order = np.lexsort((tile_of, q, rank))
    rank_s, q_s, tile_s = rank[order], q[order], tile_of[order]
    lidx_s, colrel_s, norm_s = lidx[order], col_rel[order], norm[order]

    cnt = np.zeros((NCORES, 4, NT), np.int64)
    np.add.at(cnt, (rank_s, q_s, tile_s), 1)
    nch_common = np.ceil(cnt / P).astype(np.int64).max(axis=0)   # [4, NT]
    totch = int(nch_common.sum())
    E_pad = totch * P

    core_offsets = np.concatenate([[0], np.cumsum(cnt.reshape(NCORES, -1).sum(axis=1))])
    cell_out = np.concatenate([[0], np.cumsum(nch_common.reshape(-1) * P)])[:-1].reshape(4, NT)

    g_idx = np.zeros((NCORES, E_pad), np.int64)
    g_colrel = np.zeros((NCORES, E_pad), np.float32)
    g_norm = np.zeros((NCORES, E_pad), np.float32)
    for c in range(NCORES):
        sel = slice(core_offsets[c], core_offsets[c + 1])
        li, cr, nm = lidx_s[sel], colrel_s[sel], norm_s[sel]
        qs, ts = q_s[sel], tile_s[sel]
        cell_start_c = np.concatenate([[0], np.cumsum(cnt[c].reshape(-1))])[:-1].reshape(4, NT)
        pos_in_cell = np.arange(len(li)) - cell_start_c[qs, ts]
        out_pos = cell_out[qs, ts] + pos_in_cell
        g_idx[c, out_pos] = li
        g_colrel[c, out_pos] = cr
        g_norm[c, out_pos] = nm

    # chunk -> (q, tile) and gather-op split (quadrant-pure, <= MAXCH_OP chunks)
    chunk_q = np.repeat(np.arange(4), nch_common.sum(axis=1))
    chunk_tile = np.concatenate([np.repeat(np.arange(NT), nch_common[qq]) for qq in range(4)])
    ops = []
    ch = 0
    for qq in range(4):
        q_count = int(nch_common[qq].sum())
        done = 0
        while done < q_count:
            n = int(min(MAXCH_OP, q_count - done))
            ops.append((qq, ch + done, n))
            done += n
        ch += q_count

    tot_idx_cols = sum(n * 8 for (_, _, n) in ops)   # n*128/16 cols per op
    gidx_w = np.zeros((NCORES, P, tot_idx_cols), np.int16)
    op_col = []
    colp = 0
    ar = np.arange(MAXCH_OP * P)
    for (qq, c0, n) in ops:
        nidx = n * P
        a = ar[:nidx]
        for c in range(NCORES):
            L = g_idx[c, c0 * P:(c0 + n) * P].astype(np.int16)
            blk = np.zeros((16, nidx // 16), np.int16)
            blk[a % 16, a // 16] = L
            gidx_w[c, :, colp:colp + nidx // 16] = np.tile(blk, (8, 1))
        op_col.append(colp)
        colp += nidx // 16

    selc = g_colrel.reshape(NCORES, totch, P).transpose(0, 2, 1).copy()
    seln = g_norm.reshape(NCORES, totch, P).transpose(0, 2, 1).copy()

    xT = np.zeros((NCORES, N_FEAT, NSH), np.float32)
    xs = np.asarray(x, dtype=np.float32)
    for c in range(NCORES):
        xT[c, :, :NSH_V] = xs[c * NSH_V:(c + 1) * NSH_V].T

    lab = np.asarray(labels, dtype=np.float32).reshape(-1)
    wA = np.zeros((NCORES, P, NT), np.float32)
    wB = np.zeros((NCORES, P, NT), np.float32)
    for c in range(NCORES):
        lsh = np.zeros(NSH, np.float32)
        lsh[:NSH_V] = lab[c * NSH_V:(c + 1) * NSH_V]
        msk = np.zeros(NSH, np.float32)
        msk[:NSH_V] = 1.0
        lcm = lsh.reshape(NT, P).T
        mcm = msk.reshape(NT, P).T
        wA[c] = -(mcm * lcm)
        wB[c] = -(mcm * (1.0 - lcm))

    cell_start_ch = np.concatenate([[0], np.cumsum(nch_common.reshape(-1))])[:-1].reshape(4, NT)
    meta = dict(totch=totch, ops=tuple(ops), op_col=tuple(op_col),
                nch_common=nch_common, chunk_q=chunk_q, chunk_tile=chunk_tile,
                cell_start_ch=cell_start_ch, tot_idx_cols=tot_idx_cols)
    tensors = dict(gidx=gidx_w, selc=selc, seln=seln, xT=xT, wA=wA, wB=wB)
    return meta, tensors


def _build(meta, reps=1):
    import contextlib
    import concourse.bass as bass
    import concourse.tile as tile
    from concourse import bacc, mybir

    f32 = mybir.dt.float32
    bf16 = mybir.dt.bfloat16
    i16 = mybir.dt.int16
    AF = mybir.ActivationFunctionType
    ALU = mybir.AluOpType

    totch = meta["totch"]
    ops = meta["ops"]
    op_col = meta["op_col"]
    nch_common = meta["nch_common"]
    cell_start_ch = meta["cell_start_ch"]
    chunk_tile = meta["chunk_tile"]
    chunk_q = meta["chunk_q"]
    tic = meta["tot_idx_cols"]

    nc = bacc.Bacc("TRN2", target_bir_lowering=False, debug=False, num_devices=NCORES)

    xT_in = nc.dram_tensor("xT_in", [N_FEAT, NSH], f32, kind="ExternalInput")
    gidx_in = nc.dram_tensor("gidx_in", [P, tic], i16, kind="ExternalInput")
    selc_in = nc.dram_tensor("selc_in", [P, totch], f32, kind="ExternalInput")
    seln_in = nc.dram_tensor("seln_in", [P, totch], f32, kind="ExternalInput")
    W1_in = nc.dram_tensor("W1_in", [N_FEAT, HID], f32, kind="ExternalInput")
    W2_in = nc.dram_tensor("W2_in", [HID, HID], bf16, kind="ExternalInput")
    W3_in = nc.dram_tensor("W3_in", [HID, HID], bf16, kind="ExternalInput")
    b1_in = nc.dram_tensor("b1_in", [HID, 1], f32, kind="ExternalInput")
    b2_in = nc.dram_tensor("b2_in", [HID, 1], f32, kind="ExternalInput")
    b3_in = nc.dram_tensor("b3_in", [HID, 1], f32, kind="ExternalInput")
    lW1_in = nc.dram_tensor("lW1_in", [HID, 8], bf16, kind="ExternalInput")
    lb1_in = nc.dram_tensor("lb1_in", [P, 8], f32, kind="ExternalInput")
    lW2_in = nc.dram_tensor("lW2_in", [P, 8], f32, kind="ExternalInput")
    lb2_in = nc.dram_tensor("lb2_in", [P, 2], f32, kind="ExternalInput")  # [:,0]=lb2, [:,1]=-lb2
    iota_in = nc.dram_tensor("iota_in", [P, P], f32, kind="ExternalInput")
    ident_in = nc.dram_tensor("ident_in", [P, P], f32, kind="ExternalInput")
    wA_in = nc.dram_tensor("wA_in", [P, NT], f32, kind="ExternalInput")
    wB_in = nc.dram_tensor("wB_in", [P, NT], f32, kind="ExternalInput")

    p_out = nc.dram_tensor("p_out", [NSH], f32, kind="ExternalOutput")
    partials_out = nc.dram_tensor("partials_out", [P, 2], f32, kind="ExternalOutput")

    with tile.TileContext(nc) as tc:
        with contextlib.ExitStack() as stack:
            const = stack.enter_context(tc.tile_pool(name="const", bufs=1))
            idxp = stack.enter_context(tc.tile_pool(name="idxp", bufs=1))
            hp = stack.enter_context(tc.tile_pool(name="hp", bufs=1))
            xtp = stack.enter_context(tc.tile_pool(name="xtp", bufs=3))
            mep = stack.enter_context(tc.tile_pool(name="mep", bufs=4))
            selp = stack.enter_context(tc.tile_pool(name="selp", bufs=4))
            mtp = stack.enter_context(tc.tile_pool(name="mtp", bufs=3))
            psA = stack.enter_context(tc.tile_pool(name="psA", bufs=2, space="PSUM"))
            psC = stack.enter_context(tc.tile_pool(name="psC", bufs=4, space="PSUM"))
            dram = stack.enter_context(tc.tile_pool(name="dram", bufs=1, space="DRAM"))

            iota = const.tile([P, P], f32)
            nc.sync.dma_start(iota[:], iota_in[:])
            ident = const.tile([P, P], f32)
            nc.sync.dma_start(ident[:], ident_in[:])
            W1 = const.tile([N_FEAT, HID], f32)
            nc.sync.dma_start(W1[:], W1_in[:])
            W2 = const.tile([HID, HID], bf16)
            nc.sync.dma_start(W2[:], W2_in[:])
            W3 = const.tile([HID, HID], bf16)
            nc.sync.dma_start(W3[:], W3_in[:])
            b1 = const.tile([HID, 1], f32)
            nc.sync.dma_start(b1[:], b1_in[:])
            b2 = const.tile([HID, 1], f32)
            nc.sync.dma_start(b2[:], b2_in[:])
            b3 = const.tile([HID, 1], f32)
            nc.sync.dma_start(b3[:], b3_in[:])
            lW1 = const.tile([HID, 8], bf16)
            nc.sync.dma_start(lW1[:], lW1_in[:])
            lb1r = const.tile([P, 8], f32)
            nc.sync.dma_start(lb1r[:], lb1_in[:])
            lW2r = const.tile([P, 8], f32)
            nc.sync.dma_start(lW2r[:], lW2_in[:])
            lb2t = const.tile([P, 2], f32)
            nc.sync.dma_start(lb2t[:], lb2_in[:])
            wAt = const.tile([P, NT], f32)
            nc.sync.dma_start(wAt[:], wA_in[:])
            wBt = const.tile([P, NT], f32)
            nc.sync.dma_start(wBt[:], wB_in[:])

            gidx = idxp.tile([P, tic], i16)
            nc.sync.dma_start(gidx[:], gidx_in[:])
            selc = idxp.tile([P, totch], f32)
            nc.sync.dma_start(selc[:], selc_in[:])
            seln = idxp.tile([P, totch], f32)
            nc.sync.dma_start(seln[:], seln_in[:])

            hT = [hp.tile([HID, NSH], bf16, name="hT0"),
                  hp.tile([HID, NSH], bf16, name="hT1")]
            hacc = hp.tile([HID, NSH], f32, name="hacc")

            def phase_a(layer, m_loc):
                for j in range(NT):
                    ps = psA.tile([P, HID], f32, tag="psA", space="PSUM")
                    if layer == 0:
                        xt = xtp.tile([N_FEAT, P], f32, tag="xt")
                        nc.sync.dma_start(xt[:], xT_in[:, j * P:(j + 1) * P])
                        nc.tensor.matmul(ps[:], lhsT=xt[:], rhs=W1[:],
                                         start=True, stop=True)
                    else:
                        W = W2 if layer == 1 else W3
                        nc.tensor.matmul(ps[:], lhsT=hT[(layer + 1) % 2][:, j * P:(j + 1) * P],
                                         rhs=W[:], start=True, stop=True)
                    mt = mtp.tile([P, HID], f32, tag="mt")
                    nc.vector.tensor_copy(mt[:], ps[:])
                    nc.sync.dma_start(m_loc[j * P:(j + 1) * P, :], mt[:])

            def phase_c(layer, m_full):
                first_visit = [True] * NT
                cur_psum = {}
                for oi, (qq, c0, n) in enumerate(ops):
                    me = mep.tile([P, MAXCH_OP, HID], f32, tag="me")
                    nidx = n * P
                    tbl = m_full[qq * QROWS:(qq + 1) * QROWS, :]
                    nc.gpsimd.dma_gather(me[:, :n, :], tbl,
                                         gidx[:, op_col[oi]:op_col[oi] + nidx // 16],
                                         nidx, nidx, HID)
                    for k in range(n):
                        g = c0 + k
                        t = int(chunk_tile[g])
                        q_of_g = int(chunk_q[g])
                        sel = selp.tile([P, P], f32, tag="sel")
                        nc.vector.tensor_scalar(sel[:], iota[:], selc[:, g:g + 1],
                                                seln[:, g:g + 1],
                                                ALU.is_equal, ALU.mult)
                        if t not in cur_psum:
                            cur_psum[t] = psC.tile([HID, P], f32, name="psc_t", tag="psC", space="PSUM")
                        nch_cell = int(nch_common[q_of_g][t])
                        pos = g - int(cell_start_ch[q_of_g][t])
                        nc.tensor.matmul(cur_psum[t][:], lhsT=me[:, k, :], rhs=sel[:],
                                         start=(pos == 0), stop=(pos == nch_cell - 1))
                        if pos == nch_cell - 1:
                            dst = hacc[:, t * P:(t + 1) * P]
                            if first_visit[t]:
                                nc.vector.tensor_copy(dst, cur_psum[t][:])
                                first_visit[t] = False
                            else:
                                nc.vector.tensor_add(dst, dst, cur_psum[t][:])
                            del cur_psum[t]
                b = (b1, b2, b3)[layer]
                for j in range(NT):
                    nc.scalar.activation(hT[layer % 2][:, j * P:(j + 1) * P],
                                         hacc[:, j * P:(j + 1) * P],
                                         AF.Relu, bias=b[:, 0:1])

            def head_and_loss():
                h3 = hT[0]
                z_all = hp.tile([P, NT], f32, name="z_all", tag="z_all")
                for j in range(NT):
                    psh = psA.tile([P, 8], f32, tag="psA", space="PSUM")
                    nc.tensor.matmul(psh[:], lhsT=h3[:, j * P:(j + 1) * P], rhs=lW1[:],
                                     start=True, stop=True)
                    h4 = mtp.tile([P, 8], f32, tag="h4")
                    nc.vector.tensor_add(h4[:], psh[:], lb1r[:])
                    nc.vector.tensor_scalar_max(h4[:], h4[:], 0.0)
                    nc.vector.tensor_tensor(h4[:], h4[:], lW2r[:], op=ALU.mult)
                    nc.vector.reduce_sum(z_all[:, j:j + 1], h4[:], axis=mybir.AxisListType.X)

                p_all = hp.tile([P, NT], f32, name="p_all", tag="p_all")
                nc.scalar.activation(p_all[:], z_all[:], AF.Sigmoid, bias=lb2t[:, 0:1])
                pc = hp.tile([P, NT], f32, name="pc", tag="pc")
                nc.vector.tensor_scalar(pc[:], p_all[:], 1e-7, 1.0 - 1e-7,
                                        ALU.max, ALU.min)
                lnp = hp.tile([P, NT], f32, name="lnp", tag="lnp")
                nc.scalar.activation(lnp[:], pc[:], AF.Ln)
                omp = hp.tile([P, NT], f32, name="omp", tag="omp")
                nc.vector.tensor_scalar(omp[:], pc[:], -1.0, 1.0, ALU.mult, ALU.add)
                ln1m = hp.tile([P, NT], f32, name="ln1m", tag="ln1m")
                nc.scalar.activation(ln1m[:], omp[:], AF.Ln)

                ta = hp.tile([P, NT], f32, name="ta", tag="ta")
                nc.vector.tensor_tensor(ta[:], wAt[:], lnp[:], op=ALU.mult)
                tb = hp.tile([P, NT], f32, name="tb", tag="tb")
                nc.vector.tensor_tensor(tb[:], wBt[:], ln1m[:], op=ALU.mult)
                AB = hp.tile([P, 2], f32, name="AB", tag="AB")
                nc.vector.reduce_sum(AB[:, 0:1], ta[:], axis=mybir.AxisListType.X)
                nc.vector.reduce_sum(AB[:, 1:2], tb[:], axis=mybir.AxisListType.X)
                nc.sync.dma_start(partials_out[:], AB[:])

                pst = psA.tile([NT, P], f32, tag="psA", space="PSUM")
                nc.tensor.transpose(out=pst[:], in_=p_all[:, :NT], identity=ident[:])
                pT = hp.tile([NT, P], f32, name="pT", tag="pT")
                nc.vector.tensor_copy(pT[:], pst[:])
                nc.sync.dma_start(p_out[:].rearrange("(t p) -> t p", p=P), pT[:])

            m_fulls = []
            for layer in range(3):
                m_loc = dram.tile([NSH, HID], f32, name=f"m_loc{layer}", tag=f"m_loc{layer}")
                m_full = dram.tile([NFULL, HID], f32, name=f"m_full{layer}",
                                   tag=f"m_full{layer}", addr_space="Shared")
                phase_a(layer, m_loc)
                nc.gpsimd.collective_compute(
                    "AllGather", mybir.AluOpType.bypass,
                    replica_groups=[list(range(NCORES))],
                    ins=[m_loc.opt()], outs=[m_full.opt()],
                )
                m_fulls.append(m_full)
                phase_c(layer, m_full)
            head_and_loss()

            if reps > 1:
                # timing loop: repeat all compute (no collectives) against the
                # prologue's m_full buffers; outputs rewritten identically
                with tc.For_i(0, reps - 1, 1):
                    for layer in range(3):
                        m_loc = dram.tile([NSH, HID], f32, name=f"m_locL{layer}", tag=f"m_locL{layer}")
                        phase_a(layer, m_loc)
                        phase_c(layer, m_fulls[layer])
                    head_and_loss()

    nc.compile()
    return nc


def _prep_inmaps(inputs, tensors):
    import ml_dtypes
    W1 = np.asarray(inputs["W1"], np.float32)
    W2 = np.asarray(inputs["W2"], np.float32).astype(ml_dtypes.bfloat16)
    W3 = np.asarray(inputs["W3"], np.float32).astype(ml_dtypes.bfloat16)
    b1 = np.asarray(inputs["b1"], np.float32).reshape(HID, 1)
    b2 = np.asarray(inputs["b2"], np.float32).reshape(HID, 1)
    b3 = np.asarray(inputs["b3"], np.float32).reshape(HID, 1)
    lW1 = np.asarray(inputs["lW1"], np.float32).astype(ml_dtypes.bfloat16)
    lb1 = np.broadcast_to(np.asarray(inputs["lb1"], np.float32), (P, 8)).copy()
    lW2 = np.broadcast_to(np.asarray(inputs["lW2"], np.float32).reshape(1, 8), (P, 8)).copy()
    lb2v = float(np.asarray(inputs["lb2"], np.float32).reshape(-1)[0])
    lb2 = np.broadcast_to(np.array([[lb2v, -lb2v]], np.float32), (P, 2)).copy()
    iota = np.broadcast_to(np.arange(P, dtype=np.float32), (P, P)).copy()
    ident = np.eye(P, dtype=np.float32)

    in_maps = []
    for c in range(NCORES):
        in_maps.append({
            "xT_in": tensors["xT"][c],
            "gidx_in": tensors["gidx"][c],
            "selc_in": tensors["selc"][c],
            "seln_in": tensors["seln"][c],
            "W1_in": W1, "W2_in": W2, "W3_in": W3,
            "b1_in": b1, "b2_in": b2, "b3_in": b3,
            "lW1_in": lW1, "lb1_in": lb1, "lW2_in": lW2, "lb2_in": lb2,
            "iota_in": iota, "ident_in": ident,
            "wA_in": tensors["wA"][c], "wB_in": tensors["wB"][c],
        })
    return in_maps


def _postprocess(results, labels):
    p_full = np.concatenate([results[c]["p_out"][:NSH_V] for c in range(NCORES)])
    A = sum(float(results[c]["partials_out"][:, 0].sum()) for c in range(NCORES))
    B = sum(float(results[c]["partials_out"][:, 1].sum()) for c in range(NCORES))
    y = np.asarray(labels, np.float32)
    pm = float(y.mean())
    loss = np.float32(((1.0 - pm) * A + pm * B) / N_NODES)
    return loss, p_full.reshape(N_NODES, 1).astype(np.float32)


def kernel(**inputs):
    from concourse.bass_utils import run_bass_kernel_spmd

    x = np.asarray(inputs["x"], np.float32)
    edge_index = np.asarray(inputs["edge_index"])
    labels = np.asarray(inputs["labels"])

    meta, tensors = _host_prep(x, edge_index, labels)
    key = (meta["totch"], meta["tot_idx_cols"], meta["ops"])
    if key not in _cache:
        _cache[key] = _build(meta)
    nc = _cache[key]

    in_maps = _prep_inmaps(inputs, tensors)
    res = run_bass_kernel_spmd(nc, in_maps, core_ids=list(range(NCORES)))
    return _postprocess(res.results, labels)
